# revision 1
# baseline (speedup 1.0000x reference)
"""DSS kernel on 8 trn2 cores.

out[l, h] = Re( sum_n Wk[h,n] * exp(dtLambda[h,n] * l) ),  (L=2048, H=1024)

Per-channel form: out[l,h] = sum_n A*exp(a*l)*sin(b*l + phi'),
  a = dt0[h]*Re(Lam)[n] <= 0, b = dt1[h]*Im(Lam)[n],
  A = |Wk|, phi' = atan2(Im Wk, Re Wk) + pi/2.

Sharding: H split across 8 cores (128 channels each). On-core layout:
partition p = (h2, n) with h2 in {0,1}, n in [0,64); 64 channel-pairs.

Per pair j (channels 2j, 2j+1):
 - phases: int32 fixed-point (units 2pi/2^16), geometric doubling adds
   (values < 2^19, no overflow anywhere); ACT Sin reads the low 16 bits
   via int16 bitcast + stride-2 AP (signed reinterp == exact mod 2pi,
   args in [-pi,pi) where the spline is ~2e-7 accurate).
 - E = A*exp(a*l): fp16 geometric doubling on DVE (per-partition fp32
   scalar multipliers exp(a*64*2^k)).
 - Q = E*C: fp16 tensor_tensor mult (DVE 2x mode).
 - reduce over n: TensorE matmuls with a sliding-window ones-block
   stationary (128->2 per pair), 64 pairs accumulate into one
   (128,512) PSUM tile per l-chunk; 4 chunks; DMA PSUM->DRAM.
Host does all (H,N) prep in float64 and the final (H,L)->(L,H) transpose.
"""
import math
import numpy as np

H, N, L_EXPECTED = 1024, 64, 2048
EPS = 1e-7
NCORES = 8
HC = H // NCORES          # 128 channels per core
NPAIR = HC // 2           # 64
P = 128                   # partitions
SEED = 64                 # seeded l-range for both doublings
NLEV = 5                  # 64 -> 2048
CHUNK = 512
NCHUNK = L_EXPECTED // CHUNK

_cache = {}

# pairs with (j % PH_DEN) < PH_NUM run their phase doubling on DVE, rest on
# GPSIMD. Values stay < 2^19 so DVE int32 adds are exact (no saturation,
# integers < 2^24 are exact even through an fp32 ALU path).
PH_NUM = 3
PH_DEN = 8
BUFS = 7


def _build_program():
    from contextlib import ExitStack
    from concourse import bacc, tile, mybir

    F32 = mybir.dt.float32
    F16 = mybir.dt.float16
    I32 = mybir.dt.int32
    I16 = mybir.dt.int16
    AF = mybir.ActivationFunctionType
    OP = mybir.AluOpType

    nc = bacc.Bacc("TRN2", target_bir_lowering=False, debug=False,
                   num_devices=NCORES)
    pseed_ap = nc.dram_tensor("pseed", [P, NPAIR * SEED], I32, kind="ExternalInput").ap()
    pconst_ap = nc.dram_tensor("pconst", [P, NPAIR * NLEV], I32, kind="ExternalInput").ap()
    eseed_ap = nc.dram_tensor("eseed", [P, NPAIR * SEED], F16, kind="ExternalInput").ap()
    esc_ap = nc.dram_tensor("esc", [P, NPAIR * NLEV], F32, kind="ExternalInput").ap()
    wones_ap = nc.dram_tensor("wones", [P, 256], F16, kind="ExternalInput").ap()
    out_ap = nc.dram_tensor("out_hl", [P, L_EXPECTED], F32, kind="ExternalOutput").ap()

    with tile.TileContext(nc) as tc, ExitStack() as ctx:
        const_pool = ctx.enter_context(tc.tile_pool(name="const", bufs=1))
        ph_pool = ctx.enter_context(tc.tile_pool(name="ph", bufs=BUFS))
        e_pool = ctx.enter_context(tc.tile_pool(name="e", bufs=BUFS))
        c_pool = ctx.enter_context(tc.tile_pool(name="c", bufs=BUFS))
        q_pool = ctx.enter_context(tc.tile_pool(name="q", bufs=BUFS))
        ps_pool = ctx.enter_context(tc.tile_pool(name="ps", bufs=1, space="PSUM"))

        pconst_t = const_pool.tile([P, NPAIR * NLEV], I32, tag="pconst")
        nc.sync.dma_start(pconst_t[:], pconst_ap[:])
        esc_t = const_pool.tile([P, NPAIR * NLEV], F32, tag="esc")
        nc.sync.dma_start(esc_t[:], esc_ap[:])
        wones_t = const_pool.tile([P, 256], F16, tag="wones")
        nc.sync.dma_start(wones_t[:], wones_ap[:])
        sc_t = const_pool.tile([P, 1], F32, tag="sc")
        nc.vector.memset(sc_t[:], float(2.0 * math.pi / 65536.0))

        psum_tiles = [ps_pool.tile([P, CHUNK], F32, tag=f"ps{c}", name=f"ps{c}") for c in range(NCHUNK)]

        for j in range(NPAIR):
            # ---- phases (GPSIMD int32 doubling adds) ----
            ph = ph_pool.tile([P, L_EXPECTED], I32, tag="ph")
            nc.sync.dma_start(ph[:, 0:SEED], pseed_ap[:, j * SEED:(j + 1) * SEED])
            X = SEED
            ph_eng = nc.vector if (j % PH_DEN) < PH_NUM else nc.gpsimd
            for k in range(NLEV):
                cb = pconst_t[:, j * NLEV + k: j * NLEV + k + 1].to_broadcast((P, X))
                ph_eng.tensor_tensor(ph[:, X:2 * X], ph[:, 0:X], cb, OP.add)
                X *= 2
            # ---- C = sin(phase) fp16, reading low 16 bits of int32 ----
            ph16 = ph[:].bitcast(I16)[:, 0:2 * L_EXPECTED:2]
            ct = c_pool.tile([P, L_EXPECTED], F16, tag="c")
            nc.scalar.activation(ct[:], ph16, AF.Sin, scale=sc_t[:])
            # ---- E = A*exp(a*l) fp16 doubling (DVE tensor_scalar mult) ----
            et = e_pool.tile([P, L_EXPECTED], F16, tag="e")
            nc.sync.dma_start(et[:, 0:SEED], eseed_ap[:, j * SEED:(j + 1) * SEED])
            X = SEED
            for k in range(NLEV):
                nc.vector.tensor_scalar(
                    et[:, X:2 * X], et[:, 0:X],
                    esc_t[:, j * NLEV + k: j * NLEV + k + 1], None, OP.mult)
                X *= 2
            # ---- Q = E * C (fp16, DVE 2x) ----
            qt = q_pool.tile([P, L_EXPECTED], F16, tag="q")
            nc.vector.tensor_tensor(qt[:], et[:], ct[:], OP.mult)
            # ---- reduce over n via sliding ones-block matmuls ----
            lhsT = wones_t[:, 128 - 2 * j:256 - 2 * j]
            for c in range(NCHUNK):
                nc.tensor.matmul(psum_tiles[c][:], lhsT,
                                 qt[:, c * CHUNK:(c + 1) * CHUNK],
                                 start=(j == 0), stop=(j == NPAIR - 1))
        for c in range(NCHUNK):
            ot = const_pool.tile([P, CHUNK], F32, tag=f"o{c}", name=f"o{c}")
            nc.scalar.copy(ot[:], psum_tiles[c][:])
            nc.sync.dma_start(out_ap[:, c * CHUNK:(c + 1) * CHUNK], ot[:])
    nc.compile()
    return nc


def _prep_inputs(log_dt, llnr, lim, W):
    """All f64 host prep. Returns per-core input dicts."""
    LamRe = -np.exp(llnr.astype(np.float64))          # (N,)
    LamIm = lim.astype(np.float64)                    # (N,)
    Lam = LamRe + 1j * LamIm
    dt = np.exp(log_dt.astype(np.float64))            # (H,2)
    a = dt[:, 0:1] * LamRe[None, :]                   # (H,N)
    b = dt[:, 1:2] * LamIm[None, :]                   # (H,N)
    dtL = a + 1j * b
    Wc = W[..., 0].astype(np.float64) + 1j * W[..., 1].astype(np.float64)
    norm_sq = np.maximum((Lam * np.conj(Lam)).real, EPS * EPS)
    recip = np.conj(Lam) / norm_sq
    Wk = Wc * (np.exp(dtL) - 1.0) * recip[None, :]    # (H,N) complex
    A = np.abs(Wk)
    phi = np.arctan2(Wk.imag, Wk.real) + 0.5 * np.pi  # cos -> sin shift

    lseed = np.arange(SEED, dtype=np.float64)
    in_maps = []
    for core in range(NCORES):
        # index arrays: channel[p, j] for p=(h2*64+n), pair j
        h2 = (np.arange(P) // N)[:, None]             # (P,1)
        nn = (np.arange(P) % N)[:, None]              # (P,1)
        jj = np.arange(NPAIR)[None, :]                # (1,NPAIR)
        ch = core * HC + 2 * jj + h2                  # (P,NPAIR) global channel
        a_p = a[ch, nn]                               # (P,NPAIR)
        b_p = b[ch, nn]
        A_p = A[ch, nn]
        phi_p = phi[ch, nn]

        # phase seeds / consts in 2^16 fixed point (values in [0, 2^16))
        turns = (b_p[:, :, None] * lseed[None, None, :] + phi_p[:, :, None]) / (2 * np.pi)
        pseed = np.round((turns - np.floor(turns)) * 65536.0).astype(np.int64) % 65536
        pseed = pseed.reshape(P, NPAIR * SEED).astype(np.int32)
        lev = (SEED * (2 ** np.arange(NLEV)))[None, None, :]     # (1,1,NLEV)
        tlev = b_p[:, :, None] * lev / (2 * np.pi)
        pconst = np.round((tlev - np.floor(tlev)) * 65536.0).astype(np.int64) % 65536
        pconst = pconst.reshape(P, NPAIR * NLEV).astype(np.int32)

        # E seeds fp16 + level multipliers f32
        eseed = (A_p[:, :, None] * np.exp(a_p[:, :, None] * lseed[None, None, :]))
        eseed = eseed.reshape(P, NPAIR * SEED).astype(np.float16)
        esc = np.exp(a_p[:, :, None] * lev).reshape(P, NPAIR * NLEV).astype(np.float32)

        wones = np.zeros((P, 256), np.float16)
        wones[:N, 128] = 1.0
        wones[N:, 129] = 1.0
        in_maps.append(dict(pseed=pseed, pconst=pconst, eseed=eseed,
                            esc=esc, wones=wones))
    return in_maps


def _reference_numpy(log_dt, llnr, lim, W, L):
    """f32 fallback for unexpected L (matches reference.py semantics)."""
    Lam = -np.exp(llnr.astype(np.float32)) + 1j * lim.astype(np.float32)
    Wc = W[..., 0] + 1j * W[..., 1]
    dt = np.exp(log_dt.astype(np.float32))
    dtL = dt[:, 0:1] * Lam.real + 1j * (dt[:, 1:2] * Lam.imag)
    pos = np.arange(L, dtype=np.float32)
    S = np.exp(dtL[None, :, :] * pos[:, None, None])
    norm_sq = np.maximum((Lam * np.conj(Lam)).real, np.float32(EPS * EPS))
    Wk = Wc * (np.exp(dtL) - 1.0) * (np.conj(Lam) / norm_sq)
    return np.einsum('hn,lhn->lh', Wk, S).real.astype(np.float32)


def kernel(**inputs):
    log_dt = np.asarray(inputs["log_dt"], np.float32)
    llnr = np.asarray(inputs["Lambda_log_neg_re"], np.float32)
    lim = np.asarray(inputs["Lambda_im"], np.float32)
    W = np.asarray(inputs["W"], np.float32)
    L = int(inputs["L"])

    if L != L_EXPECTED or log_dt.shape != (H, 2) or W.shape != (H, N, 2):
        return _reference_numpy(log_dt, llnr, lim, W, L)

    from concourse.bass_utils import run_bass_kernel_spmd

    if "nc" not in _cache:
        _cache["nc"] = _build_program()
    nc = _cache["nc"]

    in_maps = _prep_inputs(log_dt, llnr, lim, W)
    res = run_bass_kernel_spmd(nc, in_maps, core_ids=list(range(NCORES)))
    out_hl = np.concatenate([res.results[c]["out_hl"] for c in range(NCORES)], axis=0)
    return np.ascontiguousarray(out_hl.T).astype(np.float32)



# revision 3
# speedup vs baseline: 12.0223x; 12.0223x over previous
"""DSS kernel on 8 trn2 cores — chunked-power matmul formulation.

out[l, h] = Re( sum_n Wk[h,n] * z[h,n]^l ),  z = exp(dtLambda),
L=2048, H=1024, N=64.

Factorize l = R*c + r (R=64, M=L/R=32 chunks):
  Wk * z^l = (Wk * z^(R*c)) * z^r
so per channel h the (M, R) output block is ONE real matmul:
  out_blk = A_h @ S_h,  A_h (M, 128), S_h (128, R)
with K=128 rows = [n (64) x Re/Im (2)]:
  S_h[n, r]      =  Re(z^r),   S_h[64+n, r] =  Im(z^r)
  A_h[c, n]      =  Re(Wk z^(Rc)),  A_h[c, 64+n] = -Im(Wk z^(Rc))
Both factors are computed on host in f64 (from the f32-rounded dtLambda,
matching reference semantics) and DMA'd as fp16; the device does only:
DMA in -> 128 small matmuls (K=128, M=32, N=64) -> PSUM -> fp16 copy ->
DMA out.  Per-channel power-of-2 scaling keeps A in fp16 range; host
unscales.

Sharding: H split across 8 cores (128 channels each).  Per core the 128
channels are processed in NG=4 groups of 32; group b's outputs pack one
PSUM bank (128, 512): channel w=4i+j in group -> psum[32j:32j+32,
64i:64i+64] via PE column tiling (tile_position (0,32j)).
"""
import math
import numpy as np

H, N, L_EXPECTED = 1024, 64, 2048
EPS = 1e-7
NCORES = 8
HC = H // NCORES          # 128 channels per core
P = 128                   # partitions (= K of the matmul)
R = 64                    # moving columns per matmul (l within chunk)
M = L_EXPECTED // R       # 32 chunks = stationary columns
NG = 4                    # channel groups per core
GSZ = HC // NG            # 32 channels per group

_cache = {}


def _build_program():
    from contextlib import ExitStack
    from concourse import bacc, tile, mybir

    F32 = mybir.dt.float32
    F16 = mybir.dt.float16

    nc = bacc.Bacc("TRN2", target_bir_lowering=False, debug=False,
                   num_devices=NCORES)
    lhsT_ap = nc.dram_tensor("lhsT", [P, HC * M], F16, kind="ExternalInput").ap()
    rhs_ap = nc.dram_tensor("rhs", [P, HC * R], F16, kind="ExternalInput").ap()
    out_ap = nc.dram_tensor("out", [P, NG * 512], F16, kind="ExternalOutput").ap()

    with tile.TileContext(nc) as tc, ExitStack() as ctx:
        w_pool = ctx.enter_context(tc.tile_pool(name="w", bufs=NG))
        x_pool = ctx.enter_context(tc.tile_pool(name="x", bufs=NG))
        o_pool = ctx.enter_context(tc.tile_pool(name="o", bufs=2))
        ps_pool = ctx.enter_context(tc.tile_pool(name="ps", bufs=NG, space="PSUM"))

        for b in range(NG):
            wt = w_pool.tile([P, GSZ * M], F16, tag="w")
            nc.sync.dma_start(wt[:], lhsT_ap[:, b * GSZ * M:(b + 1) * GSZ * M])
            xt = x_pool.tile([P, GSZ * R], F16, tag="x")
            nc.sync.dma_start(xt[:], rhs_ap[:, b * GSZ * R:(b + 1) * GSZ * R])
            ps = ps_pool.tile([P, 512], F32, tag="ps")
            for w in range(GSZ):
                j, i = w & 3, w >> 2
                nc.tensor.matmul(ps[32 * j:32 * j + 32, 64 * i:64 * i + 64],
                                 wt[:, w * M:(w + 1) * M],
                                 xt[:, w * R:(w + 1) * R],
                                 start=True, stop=True,
                                 tile_position=(0, 32 * j))
            ot = o_pool.tile([P, 512], F16, tag="o")
            nc.vector.tensor_copy(ot[:], ps[:])
            nc.scalar.dma_start(out_ap[:, b * 512:(b + 1) * 512], ot[:])
    nc.compile()
    return nc


def _prep_inputs(log_dt, llnr, lim, W):
    """Host prep. f32 rounding of dtLambda matches reference; powers in f64.

    Returns (per-core input dicts, per-channel output scales (H,) f64).
    """
    # --- mimic reference's f32 arithmetic for the exponent ---
    LamRe = (-np.exp(llnr.astype(np.float32))).astype(np.float32)   # (N,)
    LamIm = lim.astype(np.float32)                                  # (N,)
    dt = np.exp(log_dt.astype(np.float32)).astype(np.float32)       # (H,2)
    dtL32 = (dt[:, 0:1] * LamRe[None, :]).astype(np.float32) \
        + 1j * (dt[:, 1:2] * LamIm[None, :]).astype(np.float32)     # (H,N) c64
    dtL = dtL32.astype(np.complex128)

    # Wk in f64 (from the f32-rounded pieces)
    Lam = LamRe.astype(np.float64) + 1j * LamIm.astype(np.float64)
    Wc = W[..., 0].astype(np.float64) + 1j * W[..., 1].astype(np.float64)
    norm_sq = np.maximum((Lam * np.conj(Lam)).real, EPS * EPS)
    recip = np.conj(Lam) / norm_sq
    Wk = Wc * (np.exp(dtL) - 1.0) * recip[None, :]                  # (H,N)

    pos_r = np.arange(R, dtype=np.float64)
    pos_c = np.float64(R) * np.arange(M, dtype=np.float64)
    B = np.exp(dtL[:, :, None] * pos_r[None, None, :])              # (H,N,R)
    A = Wk[:, :, None] * np.exp(dtL[:, :, None] * pos_c[None, None, :])  # (H,N,M)

    # per-channel power-of-2 scaling: keep max |A| around 2^11
    m = np.maximum(np.abs(A.real), np.abs(A.imag)).max(axis=(1, 2))  # (H,)
    m = np.where(m > 0, m, 1.0)
    s = np.exp2(np.floor(np.log2(m)) - 11.0)                         # (H,)
    A = A / s[:, None, None]

    in_maps = []
    for core in range(NCORES):
        ch = slice(core * HC, (core + 1) * HC)
        Ar = A.real[ch].transpose(1, 0, 2).reshape(N, HC * M)        # (64, HC*M)
        Ai = (-A.imag[ch]).transpose(1, 0, 2).reshape(N, HC * M)
        lhsT = np.concatenate([Ar, Ai], axis=0).astype(np.float16)   # (128, HC*M)
        Br = B.real[ch].transpose(1, 0, 2).reshape(N, HC * R)
        Bi = B.imag[ch].transpose(1, 0, 2).reshape(N, HC * R)
        rhs = np.concatenate([Br, Bi], axis=0).astype(np.float16)    # (128, HC*R)
        in_maps.append(dict(lhsT=np.ascontiguousarray(lhsT),
                            rhs=np.ascontiguousarray(rhs)))
    return in_maps, s


def _decode_output(res_out, scales_core):
    """(128, 2048) fp16 device dump -> (HC, L) f64-scaled f32 block."""
    v = np.asarray(res_out).reshape(4, 32, NG, 8, 64)   # [j, c, b, i, r]
    hl = v.transpose(2, 3, 0, 1, 4).reshape(HC, L_EXPECTED)  # h=32b+4i+j, l=64c+r
    return hl.astype(np.float64) * scales_core[:, None]


def _reference_numpy(log_dt, llnr, lim, W, L):
    """f32 fallback for unexpected shapes (matches reference.py semantics)."""
    Lam = -np.exp(llnr.astype(np.float32)) + 1j * lim.astype(np.float32)
    Wc = W[..., 0] + 1j * W[..., 1]
    dt = np.exp(log_dt.astype(np.float32))
    dtL = dt[:, 0:1] * Lam.real + 1j * (dt[:, 1:2] * Lam.imag)
    pos = np.arange(L, dtype=np.float32)
    S = np.exp(dtL[None, :, :] * pos[:, None, None])
    norm_sq = np.maximum((Lam * np.conj(Lam)).real, np.float32(EPS * EPS))
    Wk = Wc * (np.exp(dtL) - 1.0) * (np.conj(Lam) / norm_sq)
    return np.einsum('hn,lhn->lh', Wk, S).real.astype(np.float32)


def kernel(**inputs):
    log_dt = np.asarray(inputs["log_dt"], np.float32)
    llnr = np.asarray(inputs["Lambda_log_neg_re"], np.float32)
    lim = np.asarray(inputs["Lambda_im"], np.float32)
    W = np.asarray(inputs["W"], np.float32)
    L = int(inputs["L"])

    if L != L_EXPECTED or log_dt.shape != (H, 2) or W.shape != (H, N, 2):
        return _reference_numpy(log_dt, llnr, lim, W, L)

    from concourse.bass_utils import run_bass_kernel_spmd

    if "nc" not in _cache:
        _cache["nc"] = _build_program()
    nc = _cache["nc"]

    in_maps, s = _prep_inputs(log_dt, llnr, lim, W)
    res = run_bass_kernel_spmd(nc, in_maps, core_ids=list(range(NCORES)))
    out_hl = np.concatenate(
        [_decode_output(res.results[c]["out"], s[c * HC:(c + 1) * HC])
         for c in range(NCORES)], axis=0)                # (H, L)
    return np.ascontiguousarray(out_hl.T).astype(np.float32)
